# revision 13
# baseline (speedup 1.0000x reference)
"""Trainium2 Bass kernel for a 4-layer post-norm Transformer encoder with
ragged attention pooling (nn_CPEncoder).

Strategy: data-parallel over batch (B=256 -> 32 sequences per core x 8 cores),
weights replicated. On-device activations are kept feature-major
(xT: [128 d-partitions, d-tile, token]) so every matmul contracts over the
partition dimension without activation transposes:

  - projections:  out[f_tile, tok] = sum_kc W^T[d, f].T @ x16[d, tok]  (fp16)
  - scoresT[k,q] = kT[hd, k].T @ qT[hd, q] per (seq, head)             (fp16)
  - attnT[hd, q] = v_tm[k, hd].T @ expT[k, q]                          (fp16)
    where v_tm (token-major V) comes from lhsT = x16-slice per sequence.
  - per-token scalars (softmax denom, LN mean/rstd, pooling weights) are
    broadcast across partitions with tiny K=1 matmuls (ones-outer-product).

Precision design: the residual stream x stays pure fp32; matmuls read fp16
copies (input rounding averages out in the fp32 PSUM accumulation). The
per-token-scalar broadcasts and the pooling score matmul run in full fp32
(they feed a softmax over scores of magnitude ~10, which amplifies absolute
error ~10x). All ACT functions used (Identity/Exp/Ln/Relu/Square) live in one
activation table -> no table reloads.
"""

import math

import numpy as np

B, S, D, H, DFF, L = 256, 100, 512, 8, 2048, 4
HD = D // H
NCORES = 8
BL = B // NCORES           # 32 sequences per core
T = BL * S                 # 3200 tokens per core
KC = D // 128              # 4 d-tiles
FQK = 2 * D // 128         # 8 q+k feature tiles
FD = DFF // 128            # 16 dff tiles
SPC = 4                    # sequences per chunk
CH = SPC * S               # 400-token chunk
NCH = BL // SPC            # 8 chunks
EPS = 1e-5

_cache = {}


def _build_nc():
    import concourse.bacc as bacc
    import concourse.tile as tile
    import concourse.mybir as mybir
    from contextlib import ExitStack

    dt = mybir.dt
    AF = mybir.ActivationFunctionType
    f32, f16 = dt.float32, dt.float16

    nc = bacc.Bacc("TRN2", target_bir_lowering=False, debug=False)

    x0T_d = nc.dram_tensor("x0T", [128, KC, T], f32, kind="ExternalInput")
    winT_d = nc.dram_tensor("winT", [L, 128, KC, 3 * D], f16, kind="ExternalInput")
    woT_d = nc.dram_tensor("woT", [L, 128, KC, D], f16, kind="ExternalInput")
    w1T_d = nc.dram_tensor("w1T", [L, 128, KC, DFF], f16, kind="ExternalInput")
    w2T_d = nc.dram_tensor("w2T", [L, 128, FD, D], f16, kind="ExternalInput")
    bqk_d = nc.dram_tensor("bqk", [128, L, FQK], f32, kind="ExternalInput")
    bvrow_d = nc.dram_tensor("bvrow", [1, L, D], f16, kind="ExternalInput")
    bo_d = nc.dram_tensor("bo", [128, L, KC], f32, kind="ExternalInput")
    b1_d = nc.dram_tensor("b1", [128, L, FD], f32, kind="ExternalInput")
    b2_d = nc.dram_tensor("b2", [128, L, KC], f32, kind="ExternalInput")
    g1_d = nc.dram_tensor("g1", [128, L, KC], f32, kind="ExternalInput")
    gb1_d = nc.dram_tensor("gb1", [128, L, KC], f32, kind="ExternalInput")
    g2_d = nc.dram_tensor("g2", [128, L, KC], f32, kind="ExternalInput")
    gb2_d = nc.dram_tensor("gb2", [128, L, KC], f32, kind="ExternalInput")
    maskf_d = nc.dram_tensor("maskf", [128, BL], f32, kind="ExternalInput")
    maskh_d = nc.dram_tensor("maskh", [128, BL], f16, kind="ExternalInput")
    maskrow_d = nc.dram_tensor("maskrow", [1, T], f32, kind="ExternalInput")
    w2sum_d = nc.dram_tensor("w2sum", [128, KC], f32, kind="ExternalInput")
    outT_d = nc.dram_tensor("outT", [128, KC, BL], f32, kind="ExternalOutput")

    with ExitStack() as ctx:
        tc = ctx.enter_context(tile.TileContext(nc))

        singles = ctx.enter_context(tc.tile_pool(name="singles", bufs=1))
        wpool = ctx.enter_context(tc.tile_pool(name="w", bufs=1))
        xpool = ctx.enter_context(tc.tile_pool(name="x", bufs=1))
        x16p = ctx.enter_context(tc.tile_pool(name="x16", bufs=2))
        qkp = ctx.enter_context(tc.tile_pool(name="qk", bufs=2))
        vp = ctx.enter_context(tc.tile_pool(name="v", bufs=2))
        expp = ctx.enter_context(tc.tile_pool(name="exp", bufs=3))
        recp = ctx.enter_context(tc.tile_pool(name="rec", bufs=2))
        rbp = ctx.enter_context(tc.tile_pool(name="rb", bufs=3))
        atp = ctx.enter_context(tc.tile_pool(name="at", bufs=2))
        tp = ctx.enter_context(tc.tile_pool(name="t", bufs=2))
        sqp = ctx.enter_context(tc.tile_pool(name="sq", bufs=2))
        rowp = ctx.enter_context(tc.tile_pool(name="row", bufs=2))
        hp = ctx.enter_context(tc.tile_pool(name="h", bufs=1))
        ps_mm = ctx.enter_context(tc.tile_pool(name="psmm", bufs=2, space="PSUM"))
        ps_sc = ctx.enter_context(tc.tile_pool(name="pssc", bufs=3, space="PSUM"))
        ps_row = ctx.enter_context(tc.tile_pool(name="psrow", bufs=1, space="PSUM"))

        # ---- constants ----
        bqk = singles.tile([128, L, FQK], f32)
        nc.sync.dma_start(bqk[:], bqk_d[:])
        bo = singles.tile([128, L, KC], f32)
        nc.sync.dma_start(bo[:], bo_d[:])
        b1 = singles.tile([128, L, FD], f32)
        nc.sync.dma_start(b1[:], b1_d[:])
        b2 = singles.tile([128, L, KC], f32)
        nc.sync.dma_start(b2[:], b2_d[:])
        g1 = singles.tile([128, L, KC], f32)
        nc.sync.dma_start(g1[:], g1_d[:])
        gb1 = singles.tile([128, L, KC], f32)
        nc.sync.dma_start(gb1[:], gb1_d[:])
        g2 = singles.tile([128, L, KC], f32)
        nc.sync.dma_start(g2[:], g2_d[:])
        gb2 = singles.tile([128, L, KC], f32)
        nc.sync.dma_start(gb2[:], gb2_d[:])
        maskf = singles.tile([128, BL], f32)
        nc.sync.dma_start(maskf[:], maskf_d[:])
        maskh = singles.tile([128, BL], f16)
        nc.sync.dma_start(maskh[:], maskh_d[:])
        w2sum = singles.tile([128, KC], f32)
        nc.sync.dma_start(w2sum[:], w2sum_d[:])

        eps_t = singles.tile([1, 1], f32)
        nc.vector.memset(eps_t[:], EPS)
        ones_col = singles.tile([128, 1], f32)
        nc.vector.memset(ones_col[:], 1.0)
        ones_row = singles.tile([1, 128], f32)
        nc.vector.memset(ones_row[:], 1.0)
        ones_col16 = singles.tile([128, 1], f16)
        nc.vector.memset(ones_col16[:], 1.0)
        ones_row16 = singles.tile([1, 128], f16)
        nc.vector.memset(ones_row16[:], 1.0)
        outT_sb = singles.tile([128, KC, BL], f32)

        # ---- load x0 (feature-major), one tile per 4-sequence chunk ----
        xc = []
        for c in range(NCH):
            xt = xpool.tile([128, KC, CH], f32, tag=f"x{c}")
            nc.sync.dma_start(xt[:], x0T_d[:, :, c * CH:(c + 1) * CH])
            xc.append(xt)

        def fp16_copy(x):
            """fp16 snapshot of a [128, KC, CH] f32 chunk (one ACT op)."""
            y = x16p.tile([128, KC, CH], f16, tag="x16")
            nc.scalar.activation(y[:], x[:], AF.Identity)
            return y

        def layer_norm(x, l, g_sb, gb_sb):
            """In-place LN over the feature dim. Stats via fp16 ones-matmuls
            (input rounding averages out in the fp32 PSUM accumulation);
            per-token scalars broadcast back via fp32 K=1 matmuls."""
            z16 = fp16_copy(x)
            sums = ps_row.tile([1, CH], f32, tag="st_sum")
            sqs = ps_row.tile([1, CH], f32, tag="st_sq")
            for ft in range(KC):
                sq = sqp.tile([128, CH], f16, tag="sq")
                nc.scalar.activation(sq[:], x[:, ft, :], AF.Square)
                nc.tensor.matmul(sums[:], ones_col16[:], z16[:, ft, :],
                                 start=(ft == 0), stop=(ft == KC - 1))
                nc.tensor.matmul(sqs[:], ones_col16[:], sq[:],
                                 start=(ft == 0), stop=(ft == KC - 1))
            mu = rowp.tile([1, CH], f32, tag="mu")
            nc.scalar.activation(mu[:], sums[:], AF.Identity, scale=1.0 / D)
            ex2 = rowp.tile([1, CH], f32, tag="ex2")
            nc.scalar.activation(ex2[:], sqs[:], AF.Identity, scale=1.0 / D)
            var = rowp.tile([1, CH], f32, tag="var")
            nc.vector.tensor_mul(var[:], mu[:], mu[:])
            nc.vector.tensor_sub(var[:], ex2[:], var[:])
            # rstd = exp(-0.5*ln(var+eps)); Ln/Exp share the act table with
            # Identity/Relu/Square so no table reloads occur. var -> rstd and
            # mu -> mu*rstd in place to save SBUF rows.
            nc.scalar.activation(var[:], var[:], AF.Ln, bias=eps_t[0:1, 0:1])
            nc.scalar.activation(var[:], var[:], AF.Exp, scale=-0.5)
            nc.vector.tensor_mul(mu[:], mu[:], var[:])
            rs_b = ps_mm.tile([128, CH], f32, tag="mm")
            nc.tensor.matmul(rs_b[:], ones_row[:], var[:], start=True, stop=True)
            mrs_b = ps_mm.tile([128, CH], f32, tag="mm")
            nc.tensor.matmul(mrs_b[:], ones_row[:], mu[:], start=True, stop=True)
            for ft in range(KC):
                nc.vector.tensor_mul(x[:, ft, :], x[:, ft, :], rs_b[:])
                nc.vector.tensor_sub(x[:, ft, :], x[:, ft, :], mrs_b[:])
                nc.scalar.activation(x[:, ft, :], x[:, ft, :], AF.Identity,
                                     bias=gb_sb[:, l, ft:ft + 1],
                                     scale=g_sb[:, l, ft:ft + 1])

        def stage_a(l, c, win, wo, bvrow):
            x = xc[c]
            x16 = fp16_copy(x)
            # --- q,k projections (feature-major, fp16 out) ---
            qk = qkp.tile([128, FQK, CH], f16, tag="qk")
            for ft in range(FQK):
                ps = ps_mm.tile([128, CH], f32, tag="mm")
                for kc in range(KC):
                    nc.tensor.matmul(ps[:], win[:, kc, ft * 128:(ft + 1) * 128],
                                     x16[:, kc, :],
                                     start=(kc == 0), stop=(kc == KC - 1))
                nc.scalar.activation(qk[:, ft, :], ps[:], AF.Identity,
                                     bias=bqk[:, l, ft:ft + 1])
            # --- v projection (token-major per sequence, masked, fp16) ---
            v = vp.tile([128, SPC, D], f16, tag="v")
            for s in range(SPC):
                b = c * SPC + s
                cols = slice(s * S, (s + 1) * S)
                psv = ps_mm.tile([128, D], f32, tag="mm")
                for kc in range(KC):
                    nc.tensor.matmul(psv[0:S, :], x16[:, kc, cols],
                                     win[:, kc, 2 * D:3 * D],
                                     start=(kc == 0), stop=False)
                nc.tensor.matmul(psv[0:S, :], ones_row16[0:1, 0:S],
                                 bvrow[0:1, :], start=False, stop=True)
                nc.scalar.activation(v[0:S, s, :], psv[0:S, :], AF.Identity,
                                     scale=maskf[0:S, b:b + 1])
            # --- attention per sequence ---
            at = atp.tile([128, KC, CH], f16, tag="at")
            for s in range(SPC):
                b = c * SPC + s
                cols = slice(s * S, (s + 1) * S)
                et = expp.tile([128, H, S], f16, tag="exp")
                for h in range(H):
                    rows = slice((h % 2) * 64, (h % 2) * 64 + 64)
                    ps = ps_sc.tile([128, S], f32, tag="sc")
                    nc.tensor.matmul(ps[0:S, :], qk[rows, 4 + h // 2, cols],
                                     qk[rows, h // 2, cols],
                                     start=True, stop=True)
                    nc.scalar.activation(et[0:S, h, :], ps[0:S, :], AF.Exp,
                                         scale=1.0 / math.sqrt(HD))
                rec = recp.tile([1, 2, 4 * S], f32, tag="rec")
                recb = recp.tile([1, 2, 4 * S], f16, tag="recb")
                for h4 in range(2):
                    den = ps_row.tile([1, 4 * S], f32, tag="den")
                    nc.tensor.matmul(den[:], maskh[0:S, b:b + 1],
                                     et[0:S, h4 * 4:(h4 + 1) * 4, :],
                                     start=True, stop=True)
                    nc.vector.reciprocal(rec[0:1, h4, :], den[:])
                    nc.scalar.activation(recb[0:1, h4, :], rec[0:1, h4, :],
                                         AF.Identity)
                for h in range(H):
                    rows = slice((h % 2) * 64, (h % 2) * 64 + 64)
                    atps = ps_sc.tile([128, S], f32, tag="sc")
                    nc.tensor.matmul(atps[0:64, :], v[0:S, s, h * 64:(h + 1) * 64],
                                     et[0:S, h, :], start=True, stop=True)
                    rbps = ps_sc.tile([128, S], f32, tag="sc")
                    nc.tensor.matmul(rbps[0:64, :], ones_row16[0:1, 0:64],
                                     recb[0:1, h // 4, (h % 4) * S:(h % 4 + 1) * S],
                                     start=True, stop=True)
                    rb = rbp.tile([64, S], f32, tag="rb")
                    nc.vector.tensor_copy(rb[:], rbps[0:64, :])
                    nc.vector.tensor_mul(at[rows, h // 2, cols], atps[0:64, :],
                                         rb[:])
            # --- out projection + residual + LN1 ---
            for ft in range(KC):
                ps = ps_mm.tile([128, CH], f32, tag="mm")
                for kc in range(KC):
                    nc.tensor.matmul(ps[:], wo[:, kc, ft * 128:(ft + 1) * 128],
                                     at[:, kc, :],
                                     start=(kc == 0), stop=(kc == KC - 1))
                t = tp.tile([128, CH], f32, tag="t")
                nc.scalar.activation(t[:], ps[:], AF.Identity,
                                     bias=bo[:, l, ft:ft + 1])
                nc.vector.tensor_add(x[:, ft, :], x[:, ft, :], t[:])
            layer_norm(x, l, g1, gb1)

        def stage_b(l, c, w1, w2):
            x = xc[c]
            x16 = fp16_copy(x)
            ht = hp.tile([128, FD, CH], f16, tag="h")
            for dt_ in range(FD):
                ps = ps_mm.tile([128, CH], f32, tag="mm")
                for kc in range(KC):
                    nc.tensor.matmul(ps[:], w1[:, kc, dt_ * 128:(dt_ + 1) * 128],
                                     x16[:, kc, :],
                                     start=(kc == 0), stop=(kc == KC - 1))
                nc.scalar.activation(ht[:, dt_, :], ps[:], AF.Relu,
                                     bias=b1[:, l, dt_:dt_ + 1])
            for ft in range(KC):
                ps = ps_mm.tile([128, CH], f32, tag="mm")
                for dt_ in range(FD):
                    nc.tensor.matmul(ps[:], w2[:, dt_, ft * 128:(ft + 1) * 128],
                                     ht[:, dt_, :],
                                     start=(dt_ == 0), stop=(dt_ == FD - 1))
                t = tp.tile([128, CH], f32, tag="t")
                nc.scalar.activation(t[:], ps[:], AF.Identity,
                                     bias=b2[:, l, ft:ft + 1])
                nc.vector.tensor_add(x[:, ft, :], x[:, ft, :], t[:])
            layer_norm(x, l, g2, gb2)

        for l in range(L):
            win = wpool.tile([128, KC, 3 * D], f16, tag="win")
            nc.sync.dma_start(win[:], winT_d[l])
            bvrow = wpool.tile([1, D], f16, tag="bv")
            nc.sync.dma_start(bvrow[:], bvrow_d[0:1, l, :])
            wo = wpool.tile([128, KC, D], f16, tag="wo")
            nc.sync.dma_start(wo[:], woT_d[l])
            w1 = wpool.tile([128, KC, DFF], f16, tag="w1")
            nc.sync.dma_start(w1[:], w1T_d[l])
            w2 = wpool.tile([128, FD, D], f16, tag="w2")
            nc.sync.dma_start(w2[:], w2T_d[l])
            for c in range(NCH):
                stage_a(l, c, win, wo, bvrow)
            for c in range(NCH):
                stage_b(l, c, w1, w2)

        # ---- ragged attention pooling ----
        # softmax is shift-invariant, so the s_q (query-side) term cancels;
        # p = softmax over the valid prefix of s_v = x . w2sum. Scores have
        # magnitude ~10, so this matmul runs in full fp32.
        for c in range(NCH):
            x = xc[c]
            svps = ps_row.tile([1, CH], f32, tag="st_sum")
            for kc in range(KC):
                nc.tensor.matmul(svps[:], w2sum[:, kc:kc + 1], x[:, kc, :],
                                 start=(kc == 0), stop=(kc == KC - 1))
            p = rowp.tile([1, CH], f32, tag="sv")
            nc.scalar.activation(p[:], svps[:], AF.Identity)
            mrow = rowp.tile([1, CH], f32, tag="mrow")
            nc.sync.dma_start(mrow[:], maskrow_d[0:1, c * CH:(c + 1) * CH])
            for s in range(SPC):
                cols = slice(s * S, (s + 1) * S)
                mx = rowp.tile([1, 1], f32, tag="mx")
                nc.vector.reduce_max(mx[:], p[0:1, cols],
                                     axis=mybir.AxisListType.X)
                nmx = rowp.tile([1, 1], f32, tag="nmx")
                nc.vector.tensor_scalar_mul(nmx[:], mx[:], -1.0)
                nc.scalar.activation(p[0:1, cols], p[0:1, cols], AF.Exp,
                                     bias=nmx[0:1, 0:1])
                nc.vector.tensor_mul(p[0:1, cols], p[0:1, cols],
                                     mrow[0:1, cols])
                sm = rowp.tile([1, 1], f32, tag="sm")
                nc.vector.reduce_sum(sm[:], p[0:1, cols],
                                     axis=mybir.AxisListType.X)
                rc = rowp.tile([1, 1], f32, tag="rc")
                nc.vector.reciprocal(rc[:], sm[:])
                nc.vector.tensor_scalar_mul(p[0:1, cols], p[0:1, cols],
                                            rc[0:1, 0:1])
            pb = ps_mm.tile([128, CH], f32, tag="mm")
            nc.tensor.matmul(pb[:], ones_row[:], p[:], start=True, stop=True)
            for kc in range(KC):
                prod = sqp.tile([128, CH], f32, tag="prod")
                nc.vector.tensor_mul(prod[:], x[:, kc, :], pb[:])
                nc.vector.reduce_sum(
                    outT_sb[:, kc, c * SPC:(c + 1) * SPC],
                    prod[:].rearrange("p (s t) -> p s t", s=SPC),
                    axis=mybir.AxisListType.X)

        nc.sync.dma_start(outT_d[:], outT_sb[:])

    nc.compile()
    return nc


def _prep_host(inputs):
    f32 = np.float32
    emb = np.asarray(inputs["inputs_emb"], f32)
    msk = np.asarray(inputs["inputs"]).astype(f32)        # [B,S] 1/0
    in_w = np.asarray(inputs["in_proj_w"], f32)
    in_b = np.asarray(inputs["in_proj_b"], f32)
    out_w = np.asarray(inputs["out_proj_w"], f32)
    out_b = np.asarray(inputs["out_proj_b"], f32)
    l1w = np.asarray(inputs["lin1_w"], f32)
    l1b = np.asarray(inputs["lin1_b"], f32)
    l2w = np.asarray(inputs["lin2_w"], f32)
    l2b = np.asarray(inputs["lin2_b"], f32)
    ln1w = np.asarray(inputs["ln1_w"], f32)
    ln1b = np.asarray(inputs["ln1_b"], f32)
    ln2w = np.asarray(inputs["ln2_w"], f32)
    ln2b = np.asarray(inputs["ln2_b"], f32)
    attn_w = np.asarray(inputs["attn_w"], f32)

    pos = np.arange(S, dtype=f32)[:, None]
    div = np.arange(0, D, 2, dtype=f32) * (-math.log(100.0) / D)
    pe = np.zeros((S, D), f32)
    pe[:, 0::2] = np.sin(pos * div)
    pe[:, 1::2] = np.cos(pos * div)
    x0 = emb + pe[None]                                    # [B,S,D]

    def fm(w):  # [din,dout] -> [128, din//128, dout] feature-major tiles
        kt = w.shape[0] // 128
        return np.ascontiguousarray(
            w.reshape(kt, 128, w.shape[1]).transpose(1, 0, 2))

    shared = {
        "winT": np.stack([fm(in_w[l].T) for l in range(L)]).astype(np.float16),
        "woT": np.stack([fm(out_w[l].T) for l in range(L)]).astype(np.float16),
        "w1T": np.stack([fm(l1w[l].T) for l in range(L)]).astype(np.float16),
        "w2T": np.stack([fm(l2w[l].T) for l in range(L)]).astype(np.float16),
        "bqk": np.ascontiguousarray(
            in_b[:, :2 * D].reshape(L, FQK, 128).transpose(2, 0, 1)),
        "bvrow": np.ascontiguousarray(in_b[None, :, 2 * D:]).astype(np.float16),
        "bo": np.ascontiguousarray(
            out_b.reshape(L, KC, 128).transpose(2, 0, 1)),
        "b1": np.ascontiguousarray(
            l1b.reshape(L, FD, 128).transpose(2, 0, 1)),
        "b2": np.ascontiguousarray(
            l2b.reshape(L, KC, 128).transpose(2, 0, 1)),
        "g1": np.ascontiguousarray(
            ln1w.reshape(L, KC, 128).transpose(2, 0, 1)),
        "gb1": np.ascontiguousarray(
            ln1b.reshape(L, KC, 128).transpose(2, 0, 1)),
        "g2": np.ascontiguousarray(
            ln2w.reshape(L, KC, 128).transpose(2, 0, 1)),
        "gb2": np.ascontiguousarray(
            ln2b.reshape(L, KC, 128).transpose(2, 0, 1)),
        "w2sum": np.ascontiguousarray(
            attn_w[:, D:].sum(0).reshape(KC, 128).T),
    }

    in_maps = []
    for i in range(NCORES):
        sl = slice(i * BL, (i + 1) * BL)
        xcn = x0[sl].reshape(T, D)
        m = msk[sl]                                        # [BL,S]
        mtm = np.zeros((128, BL), f32)
        mtm[:S] = m.T
        per = {
            "x0T": np.ascontiguousarray(
                xcn.reshape(T, KC, 128).transpose(2, 1, 0)),
            "maskf": mtm,
            "maskh": mtm.astype(np.float16),
            "maskrow": np.ascontiguousarray(m.reshape(1, T)),
        }
        per.update(shared)
        in_maps.append(per)
    return in_maps


def kernel(**inputs):
    from concourse.bass_utils import run_bass_kernel_spmd

    if "nc" not in _cache:
        _cache["nc"] = _build_nc()
    nc = _cache["nc"]
    in_maps = _prep_host(inputs)
    res = run_bass_kernel_spmd(nc, in_maps, core_ids=list(range(NCORES)))
    out = np.zeros((B, 1, D), np.float32)
    for i in range(NCORES):
        oT = np.asarray(res.results[i]["outT"], np.float32)  # [128,KC,BL]
        out[i * BL:(i + 1) * BL, 0, :] = oT.transpose(2, 1, 0).reshape(BL, D)
    return out
